# revision 3
# baseline (speedup 1.0000x reference)
"""CounterPropagationNetwork forward pass on 8 Trainium2 NeuronCores.

Reference math:
    sq[b,h]   = (||x_b||^2 + ||w_h||^2) - 2*(x_b . w_h)   (fp32, this rounding order)
    winner[b] = argmin_h sqrt(max(sq, 0))   (first index on ties)
    output[b] = grossberg[:, winner[b]]

We mimic the reference's fp32 rounding sequence exactly so that the argmin —
including ties created by fp32 quantization of sq (|sq| ~ 500, ulp ~ 3e-5) —
agrees with the reference:
    t1 = fl(x2[b] + w2[h])          (ScalarE: Copy(w2*1 + x2_bias))
    n  = fl(2dot - t1) = -fl(sq)    (DVE subtract; IEEE RNE is antisymmetric)
    winner = argmax_h n             (DVE max/max_index; first-index ties)
2dot is computed exactly as 2*dot by pre-scaling x by 2 on the host (binary
scaling commutes with matmul rounding); the PE fp32 matmul with fp32 PSUM
accumulation is accurate to ~1e-7 relative (tighter than jax-on-CPU).
sqrt/clamp are monotone and never bind for this data, so they drop out.

Strategy: data-parallel over batch. Each of the 8 cores gets 1024 rows of x;
kohonen/grossberg weights are replicated. Per core:
  - PE: 2dot = (2x_shard) @ kohonen.T  (fp32, K=512 accumulated in PSUM)
  - ScalarE: t1;  DVE: subtract + row argmax via max/max_index
  - SWDGE: indirect-DMA gather of grossberg.T rows by winner index
"""
import numpy as np

import concourse.bacc as bacc
import concourse.bass as bass
import concourse.mybir as mybir
import concourse.tile as tile
from concourse.bass_utils import run_bass_kernel_spmd

F32 = mybir.dt.float32
I32 = mybir.dt.int32
U32 = mybir.dt.uint32

B = 8192          # batch
D = 512           # input size
H = 4096          # hidden (codebook) size
O = 1024          # output size
NCORES = 8
BS = B // NCORES  # batch shard per core (1024)
BT = 128          # batch tile (partition dim)
NBT = BS // BT    # batch tiles per core (8)
NT = 512          # score tile along H
NNT = H // NT     # score tiles (8)
KT = D // 128     # contraction tiles (4)

_CACHED_NC = None


def _build_nc():
    nc = bacc.Bacc("TRN2", target_bir_lowering=False, debug=False)

    xT_d = nc.declare_dram_parameter("xT", [D, BS], F32, False)   # 2*x_shard^T
    kT_d = nc.declare_dram_parameter("kT", [D, H], F32, False)
    w2b_d = nc.declare_dram_parameter("w2b", [BT, H], F32, False)
    x2_d = nc.declare_dram_parameter("x2", [BT, NBT], F32, False)
    gT_d = nc.declare_dram_parameter("gT", [H, O], F32, False)
    out_d = nc.declare_dram_parameter("out", [BS, O], F32, True)
    idx_d = nc.declare_dram_parameter("idx", [BS], I32, True)

    with tile.TileContext(nc) as tc:
        with (
            tc.tile_pool(name="wpool", bufs=1) as wpool,
            tc.tile_pool(name="mpool", bufs=2) as mpool,
            tc.tile_pool(name="tpool", bufs=4) as tpool,
            tc.tile_pool(name="spool", bufs=4) as spool,
            tc.tile_pool(name="gpool", bufs=2) as gpool,
            tc.tile_pool(name="pspool", bufs=4, space="PSUM") as pspool,
        ):
            kT_sb = wpool.tile([128, KT * H], F32)
            xT_sb = wpool.tile([128, KT * BS], F32)
            w2b_sb = wpool.tile([128, H], F32)
            x2_sb = wpool.tile([128, NBT], F32)
            for k in range(KT):
                nc.sync.dma_start(
                    out=kT_sb[:, k * H:(k + 1) * H],
                    in_=kT_d[k * 128:(k + 1) * 128, :],
                )
                nc.sync.dma_start(
                    out=xT_sb[:, k * BS:(k + 1) * BS],
                    in_=xT_d[k * 128:(k + 1) * 128, :],
                )
            nc.sync.dma_start(out=w2b_sb[:], in_=w2b_d[:])
            nc.sync.dma_start(out=x2_sb[:], in_=x2_d[:])

            for bt in range(NBT):
                m_sb = mpool.tile([128, H], F32, name=f"m_sb_{bt}", tag="m_sb")
                for nt in range(NNT):
                    ps = pspool.tile([128, NT], F32, name=f"ps_{bt}_{nt}", tag="ps")
                    for k in range(KT):
                        nc.tensor.matmul(
                            ps[:],
                            lhsT=xT_sb[:, k * BS + bt * BT: k * BS + (bt + 1) * BT],
                            rhs=kT_sb[:, k * H + nt * NT: k * H + (nt + 1) * NT],
                            start=(k == 0),
                            stop=(k == KT - 1),
                        )
                    t1 = tpool.tile([128, NT], F32, name=f"t1_{bt}_{nt}", tag="t1")
                    nc.scalar.add(
                        t1[:],
                        w2b_sb[:, nt * NT:(nt + 1) * NT],
                        x2_sb[:, bt:bt + 1],
                    )
                    nc.vector.tensor_tensor(
                        out=m_sb[:, nt * NT:(nt + 1) * NT],
                        in0=ps[:],
                        in1=t1[:],
                        op=mybir.AluOpType.subtract,
                    )
                mx = spool.tile([128, 8], F32, name=f"mx_{bt}", tag="mx")
                ix = spool.tile([128, 8], U32, name=f"ix_{bt}", tag="ix")
                nc.vector.max(mx[:], m_sb[:])
                nc.vector.max_index(ix[:], mx[:], m_sb[:])
                ixi = spool.tile([128, 1], I32, name=f"ixi_{bt}", tag="ixi")
                nc.vector.tensor_copy(ixi[:], ix[:, 0:1])

                g_sb = gpool.tile([128, O], F32, name=f"g_sb_{bt}", tag="g_sb")
                nc.gpsimd.indirect_dma_start(
                    out=g_sb[:],
                    out_offset=None,
                    in_=gT_d[:],
                    in_offset=bass.IndirectOffsetOnAxis(ap=ixi[:, :1], axis=0),
                )
                nc.sync.dma_start(
                    out=out_d[bt * BT:(bt + 1) * BT, :], in_=g_sb[:]
                )
                nc.sync.dma_start(out=idx_d[bt * BT:(bt + 1) * BT], in_=ixi[:, 0])

    nc.compile()
    return nc


def get_nc():
    global _CACHED_NC
    if _CACHED_NC is None:
        _CACHED_NC = _build_nc()
    return _CACHED_NC


def _row_norms_sq(a):
    """fp32 row norms, matching jnp.sum(a*a, axis=1) on this backend."""
    try:
        import jax.numpy as jnp

        return np.asarray(jnp.sum(jnp.asarray(a) * jnp.asarray(a), axis=1))
    except Exception:
        return np.sum(a.astype(np.float32) ** 2, axis=1, dtype=np.float32)


def make_in_maps(x, kohonen_weights, grossberg_weights):
    kT = np.ascontiguousarray(kohonen_weights.T)                  # [D, H]
    w2 = _row_norms_sq(kohonen_weights).astype(np.float32)        # [H]
    x2 = _row_norms_sq(x).astype(np.float32)                      # [B]
    w2b = np.ascontiguousarray(np.broadcast_to(w2, (BT, H)))      # [BT, H]
    gT = np.ascontiguousarray(grossberg_weights.T)                # [H, O]
    in_maps = []
    for c in range(NCORES):
        xs = x[c * BS:(c + 1) * BS]
        x2s = x2[c * BS:(c + 1) * BS]
        in_maps.append({
            "xT": np.ascontiguousarray(2.0 * xs.T),
            "kT": kT,
            "w2b": w2b,
            "x2": np.ascontiguousarray(x2s.reshape(NBT, BT).T),   # [BT, NBT]
            "gT": gT,
        })
    return in_maps


def kernel(x, kohonen_weights, grossberg_weights):
    x = np.asarray(x, dtype=np.float32)
    kohonen_weights = np.asarray(kohonen_weights, dtype=np.float32)
    grossberg_weights = np.asarray(grossberg_weights, dtype=np.float32)

    nc = get_nc()
    in_maps = make_in_maps(x, kohonen_weights, grossberg_weights)
    res = run_bass_kernel_spmd(nc, in_maps, list(range(NCORES))).results

    output = np.concatenate([res[c]["out"] for c in range(NCORES)], axis=0)
    winners = np.concatenate([res[c]["idx"] for c in range(NCORES)], axis=0)
    return output, winners.astype(np.int32)


# revision 5
# speedup vs baseline: 1.5239x; 1.5239x over previous
"""CounterPropagationNetwork forward pass on 8 Trainium2 NeuronCores.

Reference math:
    sq[b,h]   = (||x_b||^2 + ||w_h||^2) - 2*(x_b . w_h)   (fp32, this rounding order)
    winner[b] = argmin_h sqrt(max(sq, 0))   (first index on ties)
    output[b] = grossberg[:, winner[b]]

We mimic the reference-on-neuron's fp32 rounding sequence exactly so that the
argmin — including ties created by fp32 quantization of sq (|sq| ~ 500,
ulp ~ 3e-5) and by the sqrt (which halves relative gaps, creating more ties) —
agrees bit-for-bit with the reference run on this backend:
    p  = (2x) @ k^T                 (PE fp32; bit-identical to jax-on-neuron's
                                     matmul — verified; 2x pre-scaling is exact)
    t1 = fl(x2[b] + w2[h])          (ScalarE Identity-add; bit-exact IEEE add)
    s2 = fl(t1 - p) = fl(sq)        (DVE subtract)
    d  = Sqrt(s2)                   (ScalarE Sqrt; bit-identical to jnp sqrt
                                     on neuron — both use the ACT table)
    e  = -d                         (ScalarE Copy scale=-1; exact)
    winner = argmax_h e             (DVE max/max_index; first-index on ties,
                                     matching jnp.argmin)
The reference's max(sq, 0) clamp is bitwise identity here (sq >= ~400 for this
data distribution), so it drops out.

Strategy: data-parallel over batch. Each of the 8 cores gets 1024 rows of x;
kohonen/grossberg weights are replicated. Per core:
  - PE: 2dot = (2x_shard) @ kohonen.T  (fp32, K=512 accumulated in PSUM)
  - ScalarE: t1;  DVE: subtract + row argmax via max/max_index
  - SWDGE: indirect-DMA gather of grossberg.T rows by winner index
"""
import numpy as np

import concourse.bacc as bacc
import concourse.bass as bass
import concourse.mybir as mybir
import concourse.tile as tile
from concourse.bass_utils import run_bass_kernel_spmd

F32 = mybir.dt.float32
I32 = mybir.dt.int32
U32 = mybir.dt.uint32

B = 8192          # batch
D = 512           # input size
H = 4096          # hidden (codebook) size
O = 1024          # output size
NCORES = 8
BS = B // NCORES  # batch shard per core (1024)
BT = 128          # batch tile (partition dim)
NBT = BS // BT    # batch tiles per core (8)
NT = 512          # score tile along H
NNT = H // NT     # score tiles (8)
KT = D // 128     # contraction tiles (4)

_CACHED_NC = None


def _build_nc():
    nc = bacc.Bacc("TRN2", target_bir_lowering=False, debug=False)

    xT_d = nc.declare_dram_parameter("xT", [D, BS], F32, False)   # 2*x_shard^T
    kT_d = nc.declare_dram_parameter("kT", [D, H], F32, False)
    w2b_d = nc.declare_dram_parameter("w2b", [BT, H], F32, False)
    x2_d = nc.declare_dram_parameter("x2", [BT, NBT], F32, False)
    gT_d = nc.declare_dram_parameter("gT", [H, O], F32, False)
    out_d = nc.declare_dram_parameter("out", [BS, O], F32, True)
    idx_d = nc.declare_dram_parameter("idx", [BS], I32, True)

    with tile.TileContext(nc) as tc:
        with (
            tc.tile_pool(name="wpool", bufs=1) as wpool,
            tc.tile_pool(name="mpool", bufs=2) as mpool,
            tc.tile_pool(name="tpool", bufs=4) as tpool,
            tc.tile_pool(name="spool", bufs=4) as spool,
            tc.tile_pool(name="gpool", bufs=2) as gpool,
            tc.tile_pool(name="pspool", bufs=4, space="PSUM") as pspool,
        ):
            kT_sb = wpool.tile([128, KT * H], F32)
            xT_sb = wpool.tile([128, KT * BS], F32)
            w2b_sb = wpool.tile([128, H], F32)
            x2_sb = wpool.tile([128, NBT], F32)
            for k in range(KT):
                nc.sync.dma_start(
                    out=kT_sb[:, k * H:(k + 1) * H],
                    in_=kT_d[k * 128:(k + 1) * 128, :],
                )
                nc.sync.dma_start(
                    out=xT_sb[:, k * BS:(k + 1) * BS],
                    in_=xT_d[k * 128:(k + 1) * 128, :],
                )
            nc.sync.dma_start(out=w2b_sb[:], in_=w2b_d[:])
            nc.sync.dma_start(out=x2_sb[:], in_=x2_d[:])

            for bt in range(NBT):
                m_sb = mpool.tile([128, H], F32, name=f"m_sb_{bt}", tag="m_sb")
                for nt in range(NNT):
                    ps = pspool.tile([128, NT], F32, name=f"ps_{bt}_{nt}", tag="ps")
                    for k in range(KT):
                        nc.tensor.matmul(
                            ps[:],
                            lhsT=xT_sb[:, k * BS + bt * BT: k * BS + (bt + 1) * BT],
                            rhs=kT_sb[:, k * H + nt * NT: k * H + (nt + 1) * NT],
                            start=(k == 0),
                            stop=(k == KT - 1),
                        )
                    t1 = tpool.tile([128, NT], F32, name=f"t1_{bt}_{nt}", tag="t1")
                    nc.scalar.add(
                        t1[:],
                        w2b_sb[:, nt * NT:(nt + 1) * NT],
                        x2_sb[:, bt:bt + 1],
                    )
                    s2 = tpool.tile([128, NT], F32, name=f"s2_{bt}_{nt}", tag="s2")
                    nc.vector.tensor_tensor(
                        out=s2[:],
                        in0=t1[:],
                        in1=ps[:],
                        op=mybir.AluOpType.subtract,
                    )
                    d_t = tpool.tile([128, NT], F32, name=f"d_{bt}_{nt}", tag="d_t")
                    nc.scalar.sqrt(d_t[:], s2[:])
                    nc.scalar.mul(
                        m_sb[:, nt * NT:(nt + 1) * NT], d_t[:], -1.0
                    )
                mx = spool.tile([128, 8], F32, name=f"mx_{bt}", tag="mx")
                ix = spool.tile([128, 8], U32, name=f"ix_{bt}", tag="ix")
                nc.vector.max(mx[:], m_sb[:])
                nc.vector.max_index(ix[:], mx[:], m_sb[:])
                ixi = spool.tile([128, 1], I32, name=f"ixi_{bt}", tag="ixi")
                nc.vector.tensor_copy(ixi[:], ix[:, 0:1])

                g_sb = gpool.tile([128, O], F32, name=f"g_sb_{bt}", tag="g_sb")
                nc.gpsimd.indirect_dma_start(
                    out=g_sb[:],
                    out_offset=None,
                    in_=gT_d[:],
                    in_offset=bass.IndirectOffsetOnAxis(ap=ixi[:, :1], axis=0),
                )
                nc.sync.dma_start(
                    out=out_d[bt * BT:(bt + 1) * BT, :], in_=g_sb[:]
                )
                nc.sync.dma_start(out=idx_d[bt * BT:(bt + 1) * BT], in_=ixi[:, 0])

    nc.compile()
    return nc


def get_nc():
    global _CACHED_NC
    if _CACHED_NC is None:
        _CACHED_NC = _build_nc()
    return _CACHED_NC


def _row_norms_sq(a):
    """fp32 row norms, matching jnp.sum(a*a, axis=1) on this backend."""
    try:
        import jax.numpy as jnp

        return np.asarray(jnp.sum(jnp.asarray(a) * jnp.asarray(a), axis=1))
    except Exception:
        return np.sum(a.astype(np.float32) ** 2, axis=1, dtype=np.float32)


def make_in_maps(x, kohonen_weights, grossberg_weights):
    kT = np.ascontiguousarray(kohonen_weights.T)                  # [D, H]
    w2 = _row_norms_sq(kohonen_weights).astype(np.float32)        # [H]
    x2 = _row_norms_sq(x).astype(np.float32)                      # [B]
    w2b = np.ascontiguousarray(np.broadcast_to(w2, (BT, H)))      # [BT, H]
    gT = np.ascontiguousarray(grossberg_weights.T)                # [H, O]
    in_maps = []
    for c in range(NCORES):
        xs = x[c * BS:(c + 1) * BS]
        x2s = x2[c * BS:(c + 1) * BS]
        in_maps.append({
            "xT": np.ascontiguousarray(2.0 * xs.T),
            "kT": kT,
            "w2b": w2b,
            "x2": np.ascontiguousarray(x2s.reshape(NBT, BT).T),   # [BT, NBT]
            "gT": gT,
        })
    return in_maps


def kernel(x, kohonen_weights, grossberg_weights):
    x = np.asarray(x, dtype=np.float32)
    kohonen_weights = np.asarray(kohonen_weights, dtype=np.float32)
    grossberg_weights = np.asarray(grossberg_weights, dtype=np.float32)

    nc = get_nc()
    in_maps = make_in_maps(x, kohonen_weights, grossberg_weights)
    res = run_bass_kernel_spmd(nc, in_maps, list(range(NCORES))).results

    output = np.concatenate([res[c]["out"] for c in range(NCORES)], axis=0)
    winners = np.concatenate([res[c]["idx"] for c in range(NCORES)], axis=0)
    return output, winners.astype(np.int32)
